# revision 18
# baseline (speedup 1.0000x reference)
"""Dinov2 SDPA self-attention on one TRN2 chip (8 NeuronCores).

Problem: hidden_states [4, 1370, 1024], 16 heads x 64 dim, fp32.

Sharding (hybrid data/tensor parallel): core c handles batch b = c//2 and
head-group g = c%2 (8 heads = 512 hidden columns). Each core computes its
Q/K/V projections from its batch's activations and runs attention for its
8 heads; the host concatenates the per-core [1370, 512] context outputs.
No on-chip collectives needed.

Per-core algorithm (all layouts transposed so softmax reductions become
matmul contractions):
  Xt = X^T in SBUF  [1024, 1370]
  Qt = Wq_g @ Xt + bq (per-partition bias)   [512, 1370]
  Kt = Wk_g @ Xt  (K bias is softmax-invariant -> dropped exactly)
  V  = X @ Wv_g^T + bv (natural layout, bias via DVE add; softmax weights
       sum to 1 so including bv here is exact)
  per head h: ST = Kt_h^T-tiles @ Qt_h = scores^T  [S, L] (contraction d=64;
       the head pair is emitted as PE row groups 0-63/64-127, though with
       M=128 outputs the PSUM write port serializes them anyway)
  P^T = exp(ST/8) (ACT, fused 1/sqrt(d) scale; no max-subtraction -- scores
       are bounded ~|4|, exp is safe)
  ctxT_ext = [V_h | 1]^T-style stationary @ P^T = [ctx^T; rowsums]  [65, L]
  PE-transpose 128-col slices -> [L_tile, 65], normalize by column 64 via
  DVE reciprocal + per-partition tensor_scalar multiply -> out staging.

Matmul operands are bf16 (fp32 PSUM accumulation); the ctx eviction and
transposes are bf16 too (rounds ctx and rowsums independently: measured
5.7e-3 vs 3.5e-3 with fp32 eviction -- both far under the gate, and bf16
halves the PE transpose cost). fp8/DoubleRow variants measured 2.4e-2..4.7e-2
relative error (quantization noise in a K-dim dot product grows with
sqrt(K) like the signal) -- over the 2e-2 gate, so bf16 throughout.
Validated vs fp32 reference: 5.7e-3 of absmax.

Scheduling (the kernel is exp-bound on hardware, so ACT idle is the enemy):
  - head-pair 0's K projections (all chunks) + Q (chunk 0) are emitted
    first and gate the first exp. wq/wk are hp-major in DRAM so only
    head-pair 0's 256KB slices sit on the startup DMA path (the other
    three head-pairs' slices, first read ~35us in, load later); xt chunks
    lead all three queues; wv trails.
  - all other producer work is slot-pipelined into the attention stream,
    one unit per S-tile slot, emitted at the TOP of the slot (a read takes
    no dependency on a later-emitted write): V tiles 1..10 + Q1 fill
    head-pair 0's first chunk (one-slot lookahead so each V eviction lands
    before the ctx matmul that reads it); each later head-pair's six
    projections spread into the previous head-pair's last chunk.
  - the transpose+normalize units of each chunk are deferred into the NEXT
    chunk's S-tile loop, and each chunk's FIRST score matmuls are emitted
    ahead of the previous chunk's last exp, so the hp/chunk-boundary
    serial chain never idles ACT; the final chunk's normalize units flush
    through the freed attention banks.
  - output DMAs for L-tiles 0-7 are emitted inside head-pair 3's last
    chunk (each right after its final normalize unit has drained), so
    their DGE configs and transfers overlap compute; only tiles 8-10
    remain in the kernel tail. Startup emits K-c0 + Q-c0 (the first
    exp's only gates) before V0/K-c1/K-c2.
"""

import os

import numpy as np
import ml_dtypes

import concourse.mybir as mybir
import concourse.tile as tile
from concourse import bacc
from concourse import bass_utils
from concourse.masks import make_identity

F32 = mybir.dt.float32
DT = mybir.dt.bfloat16
NPDT = ml_dtypes.bfloat16
AF = mybir.ActivationFunctionType

B = 4
L = 1370
HID = 1024
NH = 8            # heads per core
D = 64
QD = NH * D       # 512 projected dims per core
HP = NH // 2      # head pairs (PE row-group packing)
KC = HID // 128   # contraction chunks for projections

L_CHUNKS = [(0, 512), (512, 512), (1024, 346)]                      # moving/free dim
TILES = [(i * 128, min(128, L - i * 128)) for i in range((L + 127) // 128)]
NS = len(TILES)   # 11 (last tile 90)


def _body(nc, tc, xt_d, wq_d, wk_d, wv_d, bq_d, bv_d, out_d):
    with tc.tile_pool(name="persist", bufs=1) as pp:
        xt = pp.tile([128, KC, L], DT)
        wq = pp.tile([128, HP, KC, 128], DT)
        wk = pp.tile([128, HP, KC, 128], DT)
        wv = pp.tile([128, KC, QD], DT)
        qt = pp.tile([128, HP, L], DT)
        kt = pp.tile([128, HP, L], DT)
        vv = pp.tile([128, NS, NH, D + 1], DT)   # V tiles + ones column
        ost = pp.tile([128, NS, QD], F32)        # output staging, natural layout
        bqc = pp.tile([128, HP], F32)
        bvb = pp.tile([128, QD], F32)
        ident = pp.tile([128, 128], F32)
        identb = pp.tile([128, 128], DT)

        make_identity(nc, ident[:, :])
        make_identity(nc, identb[:, :])
        nc.vector.memset(vv[:, :, :, D:D + 1], 1.0)

        # Input DMAs: the startup K/Q projections need every xt chunk plus
        # wk/wq, so those are chunked and spread across the three DGE queues
        # with the gating chunks first; wv follows (first needed by the
        # V-projection phase a few microseconds in).
        xtr = [slice(k * 128, (k + 1) * 128) for k in range(KC)]

        def half(t_d, lo, hi):
            return t_d[lo * 128:hi * 128, :].rearrange("(k p) n -> p k n", p=128)

        # wq/wk are hp-major in DRAM ([HP*HID, 128]): head-pair 0's
        # 256KB slices load first (they gate the first exp); the other three
        # head-pairs' slices (first read ~35us in) are deferred.
        nc.gpsimd.dma_start(wk[:, 0, :, :],
                            wk_d[0:HID, :].rearrange("(p k) n -> p k n", p=128))
        nc.sync.dma_start(xt[:, 0, :], xt_d[xtr[0], :])
        nc.scalar.dma_start(xt[:, 1, :], xt_d[xtr[1], :])
        nc.sync.dma_start(xt[:, 3, :], xt_d[xtr[3], :])
        nc.scalar.dma_start(xt[:, 4, :], xt_d[xtr[4], :])
        nc.sync.dma_start(xt[:, 6, :], xt_d[xtr[6], :])
        nc.scalar.dma_start(xt[:, 7, :], xt_d[xtr[7], :])
        nc.gpsimd.dma_start(xt[:, 2, :], xt_d[xtr[2], :])
        nc.gpsimd.dma_start(xt[:, 5, :], xt_d[xtr[5], :])
        nc.scalar.dma_start(wq[:, 0, :, :],
                            wq_d[0:HID, :].rearrange("(p k) n -> p k n", p=128))
        nc.gpsimd.dma_start(bqc[:, :], bq_d.rearrange("(h p) o -> p (h o)", p=128))
        nc.gpsimd.dma_start(bvb[:, :], bv_d[:, :])
        nc.sync.dma_start(wv[:, 0:4, :], half(wv_d, 0, 4))
        nc.scalar.dma_start(wv[:, 4:KC, :], half(wv_d, 4, KC))
        nc.sync.dma_start(wk[:, 1:HP, :, :], wk_d[HID:HP * HID, :].rearrange(
            "(h p k) n -> p h k n", p=128, k=KC))
        nc.scalar.dma_start(wq[:, 1:HP, :, :], wq_d[HID:HP * HID, :].rearrange(
            "(h p k) n -> p h k n", p=128, k=KC))

        # ---- fused projection + attention ----
        # One concurrent PSUM layout (8 banks: pq 1 + stAB 2x2 + cAB 2 + tr 1)
        # so Q/K projections for later head pairs and the PE-transposes hide
        # inside the exp-bound attention window of earlier head pairs.
        with (
            tc.tile_pool(name="pqp", bufs=1, space="PSUM") as pqp,
            tc.tile_pool(name="sps", bufs=2, space="PSUM") as sps,
            tc.tile_pool(name="cps", bufs=1, space="PSUM") as cps,
            tc.tile_pool(name="tps", bufs=1, space="PSUM") as tps,
            tc.tile_pool(name="wp", bufs=3) as wp,
        ):
            # Head pipelining: the attention banks are idle until hp0's
            # attention starts, so early projection groups rotate through
            # them instead of serializing on the single proj slot.
            EARLY = ((pqp, "pq"), (sps, "stAB"), (cps, "cAB"), (tps, "tr"))
            early_i = 0

            def proj_unit(kind, hp, ci, pl, tg):
                l0, ln = L_CHUNKS[ci]
                m = slice(hp * 128, (hp + 1) * 128)
                if kind == "q":
                    qps = pl.tile([128, 512], F32, name="qps", tag=tg)
                    for k in range(KC):
                        nc.tensor.matmul(qps[:, :ln], wq[:, hp, k, :],
                                         xt[:, k, l0:l0 + ln],
                                         start=(k == 0), stop=(k == KC - 1))
                    nc.vector.tensor_scalar_add(qt[:, hp, l0:l0 + ln], qps[:, :ln],
                                                bqc[:, hp:hp + 1])
                else:
                    kps = pl.tile([128, 512], F32, name="kps", tag=tg)
                    for k in range(KC):
                        nc.tensor.matmul(kps[:, :ln], wk[:, hp, k, :],
                                         xt[:, k, l0:l0 + ln],
                                         start=(k == 0), stop=(k == KC - 1))
                    nc.vector.tensor_copy(kt[:, hp, l0:l0 + ln], kps[:, :ln])

            # Startup: the first exp gates on K-c0 + Q-c0 only, so emit
            # those two first (PE is in-order: anything emitted between
            # them and the first score matmul delays the first exp). V0
            # (needed by the first ctx matmul) follows, then K-c1/K-c2
            # whose real deadlines are S-tiles 4 and 8.
            proj_unit("k", 0, 0, *EARLY[0])
            proj_unit("q", 0, 0, *EARLY[1])

            def v_tile(si, pl, tg):
                s0, ss = TILES[si]
                vps = pl.tile([128, QD], F32, name="vps", tag=tg)
                for k in range(KC):
                    nc.tensor.matmul(vps[:ss, :], xt[:, k, s0:s0 + ss], wv[:, k, :],
                                     start=(k == 0), stop=(k == KC - 1))
                nc.vector.tensor_add(
                    vv[:ss, si, :, 0:D],
                    vps[:ss, :].rearrange("p (h d) -> p h d", h=NH),
                    bvb[:ss, :].rearrange("p (h d) -> p h d", h=NH),
                )

            # V tile 0 right behind the startup projections; tiles 1..10 are
            # pipelined one-per-S-tile-slot into head-pair 0's first chunk
            # with one-slot lookahead (tile si+1 is issued in slot si, so its
            # eviction lands well before the ctx matmul that reads it). hp0's
            # Q1/Q2 lead the later chunks the same way. All of these rotate
            # on the proj bank, whose chain pace matches the ACT slot pace.
            v_tile(0, *EARLY[2])
            proj_unit("k", 0, 1, *EARLY[3])
            proj_unit("k", 0, 2, *EARLY[0])

            def pu(kind, h, ci):
                return lambda: proj_unit(kind, h, ci, pqp, "pq")

            # Slot-pipelined deferred work: V tiles 1..10 plus hp0's Q1 fill
            # chunk 0's slots; each later head-pair's six projections spread
            # into the previous head-pair's last chunk. Every unit lands
            # well before its first reader.
            slot_work = {
                (0, 0): [(lambda s: lambda: v_tile(s, pqp, "pq"))(s)
                         for s in range(1, NS)] + [pu("q", 0, 1)],
                (0, 1): [pu("q", 0, 2)],
            }
            for hp in range(1, HP):
                slot_work[(hp - 1, 2)] = [
                    pu("k", hp, 0), pu("k", hp, 1), pu("k", hp, 2),
                    pu("q", hp, 0), pu("q", hp, 1), pu("q", hp, 2)]

            def emit_scores(hp, ci, si):
                l0, ln = L_CHUNKS[ci]
                s0, ss = TILES[si]
                stAB = sps.tile([128, 2, 512], F32, name="stAB", tag="stAB")
                nc.tensor.matmul(stAB[:ss, 0, :ln], kt[0:64, hp, s0:s0 + ss],
                                 qt[0:64, hp, l0:l0 + ln],
                                 start=True, stop=True, tile_position=(0, 0))
                nc.tensor.matmul(stAB[:ss, 1, :ln], kt[64:128, hp, s0:s0 + ss],
                                 qt[64:128, hp, l0:l0 + ln],
                                 start=True, stop=True, tile_position=(64, 0))
                return stAB

            backlog = []
            pre_scored = {}
            CH = [(hp, ci) for hp in range(HP) for ci in range(len(L_CHUNKS))]
            for ch_i, (hp, ci) in enumerate(CH):
                hA, hB = 2 * hp, 2 * hp + 1
                if True:
                    l0, ln = L_CHUNKS[ci]
                    slots = list(slot_work.get((hp, ci), []))
                    cAB = cps.tile([65, 2, 512], F32, name="cAB", tag="cAB")
                    for si, (s0, ss) in enumerate(TILES):
                        # Early output DMAs: L-tiles 0-3 are fully normalized
                        # once (hp3, c1) drained (hp3, c0)'s backlog, and
                        # tiles 4-7 once (hp3, c2)'s slots 0-7 drained
                        # (hp3, c1)'s -- emitting their out-DMAs here (on the
                        # idle sync/gpsimd queues, off the ACT sequencer)
                        # overlaps the DGE configs and transfers with the
                        # final chunk's compute, shrinking the kernel tail to
                        # just tiles 8-10.
                        if hp == HP - 1 and ci == 2 and si in (0, 9):
                            for ti in range(4) if si == 0 else range(4, 8):
                                t0_, tn_ = TILES[ti]
                                q = nc.sync if ti % 2 == 0 else nc.gpsimd
                                q.dma_start(out_d[t0_:t0_ + tn_, :],
                                            ost[:tn_, ti, :])
                        # One pipelined projection/V unit per S-tile slot,
                        # emitted BEFORE the slot's score matmuls (a chunk's
                        # Q projection must precede the first score matmul
                        # that reads it -- reads take no dependency on
                        # later-emitted writes).
                        if slots:
                            slots.pop(0)()
                        if si == 0 and (hp, ci) in pre_scored:
                            stAB = pre_scored.pop((hp, ci))
                        else:
                            stAB = emit_scores(hp, ci, si)
                        # ... and one deferred transpose+normalize unit from
                        # the previous chunk: keeps the hp/chunk boundary
                        # chain off ACT's critical path.
                        if backlog:
                            backlog.pop(0)(tps, "tr")
                        if si == NS - 1 and ch_i + 1 < len(CH):
                            # Software-pipeline the chunk boundary: the next
                            # chunk's first score matmuls go ahead of this
                            # chunk's last exp, so ACT never waits for them.
                            # (Their qt/kt producers were slot-emitted at
                            # least one chunk earlier.)
                            pre_scored[CH[ch_i + 1]] = emit_scores(
                                *CH[ch_i + 1], 0)
                        eAB = wp.tile([128, 2, 512], DT, name="eAB", tag="eAB")
                        nc.scalar.activation(eAB[:ss, :, :ln], stAB[:ss, :, :ln],
                                             AF.Exp, scale=0.125)
                        nc.tensor.matmul(cAB[:, 0, :ln], vv[:ss, si, hA, :],
                                         eAB[:ss, 0, :ln],
                                         start=(si == 0), stop=(si == NS - 1))
                        nc.tensor.matmul(cAB[:, 1, :ln], vv[:ss, si, hB, :],
                                         eAB[:ss, 1, :ln],
                                         start=(si == 0), stop=(si == NS - 1))
                    ctAB = wp.tile([65, 2, 512], DT, name="ctAB", tag="ctAB")
                    nc.vector.tensor_copy(ctAB[:, :, :ln], cAB[:, :, :ln])

                    def norm_unit(ctAB, l0, ln, j, h2, h):
                        def f(pl, tg):
                            lt = (l0 + j) // 128
                            w = min(128, ln - j)
                            tr = pl.tile([128, 65], DT, name="tr", tag=tg)
                            nc.tensor.transpose(tr[:w, :], ctAB[:, h2, j:j + w],
                                                identb[0:65, 0:65])
                            rc = wp.tile([128, 1], F32, name="rc", tag="rc")
                            nc.vector.reciprocal(rc[:w, :], tr[:w, 64:65])
                            nc.vector.tensor_scalar_mul(
                                ost[:w, lt, h * D:(h + 1) * D],
                                tr[:w, 0:D], rc[:w, :])
                        return f

                    for j in range(0, ln, 128):
                        for h2, h in ((0, hA), (1, hB)):
                            backlog.append(norm_unit(ctAB, l0, ln, j, h2, h))

            # Flush the final chunk's normalize units through the now-free
            # attention banks.
            for i, f in enumerate(backlog):
                pl, tg = ((tps, "tr"), (pqp, "pq"),
                          (cps, "cAB"), (sps, "stAB"))[i % 4]
                f(pl, tg)
            backlog.clear()

            # Tiles 0-7 were emitted early (inside (hp3, c2)'s slot loop);
            # only the final chunk's tiles remain.
            oqs = [nc.sync, nc.scalar, nc.gpsimd]
            for ti in range(8, NS):
                t0, tn = TILES[ti]
                oqs[ti % 3].dma_start(out_d[t0:t0 + tn, :], ost[:tn, ti, :])


_NC_CACHE = {}


def _build(reps=1):
    key = ("nc", reps)
    if key in _NC_CACHE:
        return _NC_CACHE[key]
    nc = bacc.Bacc("TRN2", target_bir_lowering=False, debug=False)
    xt_d = nc.dram_tensor("xt", [HID, L], DT, kind="ExternalInput")
    wq_d = nc.dram_tensor("wqt", [HP * HID, 128], DT, kind="ExternalInput")
    wk_d = nc.dram_tensor("wkt", [HP * HID, 128], DT, kind="ExternalInput")
    wv_d = nc.dram_tensor("wvt", [HID, QD], DT, kind="ExternalInput")
    bq_d = nc.dram_tensor("bq", [QD, 1], F32, kind="ExternalInput")
    bv_d = nc.dram_tensor("bvb", [128, QD], F32, kind="ExternalInput")
    out_d = nc.dram_tensor("out", [L, QD], F32, kind="ExternalOutput")

    with tile.TileContext(nc) as tc:
        for _ in range(reps):
            _body(nc, tc, xt_d.ap(), wq_d.ap(), wk_d.ap(), wv_d.ap(),
                  bq_d.ap(), bv_d.ap(), out_d.ap())
    nc.compile()
    _NC_CACHE[key] = nc
    return nc


def make_in_maps(hidden_states, Wq, bq, Wk, bk, Wv, bv):
    in_maps = []
    for c in range(8):
        b, g = divmod(c, 2)
        gs = slice(g * QD, (g + 1) * QD)
        in_maps.append({
            "xt": np.ascontiguousarray(hidden_states[b].T).astype(NPDT),
            "wqt": np.ascontiguousarray(
                Wq[gs, :].T.reshape(KC, 128, HP, 128).transpose(2, 1, 0, 3)
                .reshape(HP * HID, 128)).astype(NPDT),
            "wkt": np.ascontiguousarray(
                Wk[gs, :].T.reshape(KC, 128, HP, 128).transpose(2, 1, 0, 3)
                .reshape(HP * HID, 128)).astype(NPDT),
            "wvt": np.ascontiguousarray(Wv[gs, :].T).astype(NPDT),
            "bq": bq[gs].reshape(QD, 1).astype(np.float32),
            "bvb": np.ascontiguousarray(
                np.broadcast_to(bv[gs], (128, QD))).astype(np.float32),
        })
    return in_maps


LAST_RESULTS = None


def kernel(hidden_states, Wq, bq, Wk, bk, Wv, bv):
    global LAST_RESULTS
    nc = _build()
    in_maps = make_in_maps(hidden_states, Wq, bq, Wk, bk, Wv, bv)
    try:
        res = bass_utils.run_bass_kernel_spmd(
            nc, in_maps, core_ids=list(range(8)),
            trace=bool(os.environ.get("KERNEL_TRACE")),
        )
    except (ImportError, ModuleNotFoundError):
        # The axon NTFF profiling hook is absent in some containers; retry
        # with tracing disabled rather than failing the run.
        prev = os.environ.get("BASS_NEVER_TRACE")
        os.environ["BASS_NEVER_TRACE"] = "1"
        try:
            res = bass_utils.run_bass_kernel_spmd(
                nc, in_maps, core_ids=list(range(8)))
        finally:
            if prev is None:
                os.environ.pop("BASS_NEVER_TRACE", None)
            else:
                os.environ["BASS_NEVER_TRACE"] = prev
    LAST_RESULTS = res
    out = np.empty((B, L, HID), np.float32)
    for c, om in enumerate(res.results):
        b, g = divmod(c, 2)
        out[b, :, g * QD:(g + 1) * QD] = om["out"]
    return out



# revision 19
# speedup vs baseline: 1.1284x; 1.1284x over previous
"""Dinov2 SDPA self-attention on one TRN2 chip (8 NeuronCores).

Problem: hidden_states [4, 1370, 1024], 16 heads x 64 dim, fp32.

Sharding (hybrid data/tensor parallel): core c handles batch b = c//2 and
head-group g = c%2 (8 heads = 512 hidden columns). Each core computes its
Q/K/V projections from its batch's activations and runs attention for its
8 heads; the host concatenates the per-core [1370, 512] context outputs.
No on-chip collectives needed.

Per-core algorithm (all layouts transposed so softmax reductions become
matmul contractions):
  Xt = X^T in SBUF  [1024, 1370]
  Qt = Wq_g @ Xt + bq (per-partition bias)   [512, 1370]
  Kt = Wk_g @ Xt  (K bias is softmax-invariant -> dropped exactly)
  V  = X @ Wv_g^T + bv (natural layout, bias via DVE add; softmax weights
       sum to 1 so including bv here is exact)
  per head h: ST = Kt_h^T-tiles @ Qt_h = scores^T  [S, L] (contraction d=64;
       the head pair is emitted as PE row groups 0-63/64-127, though with
       M=128 outputs the PSUM write port serializes them anyway)
  P^T = exp(ST/8) (ACT, fused 1/sqrt(d) scale; no max-subtraction -- scores
       are bounded ~|4|, exp is safe)
  ctxT_ext = [V_h | 1]^T-style stationary @ P^T = [ctx^T; rowsums]  [65, L]
  PE-transpose 128-col slices -> [L_tile, 65], normalize by column 64 via
  DVE reciprocal + per-partition tensor_scalar multiply -> out staging.

Matmul operands are bf16 (fp32 PSUM accumulation); the ctx eviction and
transposes are bf16 too (rounds ctx and rowsums independently: measured
5.7e-3 vs 3.5e-3 with fp32 eviction -- both far under the gate, and bf16
halves the PE transpose cost). fp8/DoubleRow variants measured 2.4e-2..4.7e-2
relative error (quantization noise in a K-dim dot product grows with
sqrt(K) like the signal) -- over the 2e-2 gate, so bf16 throughout.
Validated vs fp32 reference: 5.7e-3 of absmax.

Scheduling (the kernel is exp-bound on hardware, so ACT idle is the enemy):
  - head-pair 0's K projections (all chunks) + Q (chunk 0) are emitted
    first and gate the first exp. wq/wk are hp-major in DRAM so only
    head-pair 0's 256KB slices sit on the startup DMA path (the other
    three head-pairs' slices, first read ~35us in, load later); xt chunks
    lead all three queues; wv trails.
  - all other producer work is slot-pipelined into the attention stream,
    one unit per S-tile slot, emitted at the TOP of the slot (a read takes
    no dependency on a later-emitted write): V tiles 1..10 + Q1 fill
    head-pair 0's first chunk (one-slot lookahead so each V eviction lands
    before the ctx matmul that reads it); each later head-pair's six
    projections spread into the previous head-pair's last chunk.
  - the transpose+normalize units of each chunk are deferred into the NEXT
    chunk's S-tile loop, and each chunk's FIRST score matmuls are emitted
    ahead of the previous chunk's last exp, so the hp/chunk-boundary
    serial chain never idles ACT; the final chunk's normalize units flush
    through the freed attention banks.
  - output DMAs for L-tiles 0-7 are emitted inside head-pair 3's last
    chunk (each right after its final normalize unit has drained), so
    their DGE configs and transfers overlap compute; only tiles 8-10
    remain in the kernel tail. Startup emits K-c0 + Q-c0 (the first
    exp's only gates) before V0/K-c1/K-c2.
"""

import os

import numpy as np
import ml_dtypes

import concourse.mybir as mybir
import concourse.tile as tile
from concourse import bacc
from concourse import bass_utils
from concourse.masks import make_identity

F32 = mybir.dt.float32
DT = mybir.dt.bfloat16
NPDT = ml_dtypes.bfloat16
AF = mybir.ActivationFunctionType

B = 4
L = 1370
HID = 1024
NH = 8            # heads per core
D = 64
QD = NH * D       # 512 projected dims per core
HP = NH // 2      # head pairs (PE row-group packing)
KC = HID // 128   # contraction chunks for projections

L_CHUNKS = [(0, 512), (512, 512), (1024, 346)]                      # moving/free dim
TILES = [(i * 128, min(128, L - i * 128)) for i in range((L + 127) // 128)]
NS = len(TILES)   # 11 (last tile 90)


def _body(nc, tc, xt_d, wq_d, wk_d, wv_d, bq_d, bv_d, out_d):
    with tc.tile_pool(name="persist", bufs=1) as pp:
        xt = pp.tile([128, KC, L], DT)
        wq = pp.tile([128, HP, KC, 128], DT)
        wk = pp.tile([128, HP, KC, 128], DT)
        wv = pp.tile([128, KC, QD], DT)
        qt = pp.tile([128, HP, L], DT)
        kt = pp.tile([128, HP, L], DT)
        vv = pp.tile([128, NS, NH, D + 1], DT)   # V tiles + ones column
        ost = pp.tile([128, NS, QD], F32)        # output staging, natural layout
        bqc = pp.tile([128, HP], F32)
        bvb = pp.tile([128, QD], F32)
        ident = pp.tile([128, 128], F32)
        identb = pp.tile([128, 128], DT)

        make_identity(nc, ident[:, :])
        make_identity(nc, identb[:, :])
        nc.vector.memset(vv[:, :, :, D:D + 1], 1.0)

        # Input DMAs: the startup K/Q projections need every xt chunk plus
        # wk/wq, so those are chunked and spread across the three DGE queues
        # with the gating chunks first; wv follows (first needed by the
        # V-projection phase a few microseconds in).
        xtr = [slice(k * 128, (k + 1) * 128) for k in range(KC)]

        def half(t_d, lo, hi):
            return t_d[lo * 128:hi * 128, :].rearrange("(k p) n -> p k n", p=128)

        # wq/wk are hp-major in DRAM ([HP*HID, 128]): head-pair 0's
        # 256KB slices load first (they gate the first exp); the other three
        # head-pairs' slices (first read ~35us in) are deferred.
        nc.gpsimd.dma_start(wk[:, 0, :, :],
                            wk_d[0:HID, :].rearrange("(p k) n -> p k n", p=128))
        nc.sync.dma_start(xt[:, 0, :], xt_d[xtr[0], :])
        nc.scalar.dma_start(xt[:, 1, :], xt_d[xtr[1], :])
        nc.sync.dma_start(xt[:, 3, :], xt_d[xtr[3], :])
        nc.scalar.dma_start(xt[:, 4, :], xt_d[xtr[4], :])
        nc.sync.dma_start(xt[:, 6, :], xt_d[xtr[6], :])
        nc.scalar.dma_start(xt[:, 7, :], xt_d[xtr[7], :])
        nc.gpsimd.dma_start(xt[:, 2, :], xt_d[xtr[2], :])
        nc.gpsimd.dma_start(xt[:, 5, :], xt_d[xtr[5], :])
        nc.scalar.dma_start(wq[:, 0, :, :],
                            wq_d[0:HID, :].rearrange("(p k) n -> p k n", p=128))
        nc.gpsimd.dma_start(bqc[:, :], bq_d.rearrange("(h p) o -> p (h o)", p=128))
        nc.gpsimd.dma_start(bvb[:, :], bv_d[:, :])
        nc.sync.dma_start(wv[:, 0:4, :], half(wv_d, 0, 4))
        nc.scalar.dma_start(wv[:, 4:KC, :], half(wv_d, 4, KC))
        nc.sync.dma_start(wk[:, 1:HP, :, :], wk_d[HID:HP * HID, :].rearrange(
            "(h p k) n -> p h k n", p=128, k=KC))
        nc.scalar.dma_start(wq[:, 1:HP, :, :], wq_d[HID:HP * HID, :].rearrange(
            "(h p k) n -> p h k n", p=128, k=KC))

        # ---- fused projection + attention ----
        # One concurrent PSUM layout (8 banks: pq 1 + stAB 2x2 + cAB 2 + tr 1)
        # so Q/K projections for later head pairs and the PE-transposes hide
        # inside the exp-bound attention window of earlier head pairs.
        with (
            tc.tile_pool(name="pqp", bufs=1, space="PSUM") as pqp,
            tc.tile_pool(name="sps", bufs=2, space="PSUM") as sps,
            tc.tile_pool(name="cps", bufs=1, space="PSUM") as cps,
            tc.tile_pool(name="tps", bufs=1, space="PSUM") as tps,
            tc.tile_pool(name="wp", bufs=3) as wp,
        ):
            # Head pipelining: the attention banks are idle until hp0's
            # attention starts, so early projection groups rotate through
            # them instead of serializing on the single proj slot.
            EARLY = ((pqp, "pq"), (sps, "stAB"), (cps, "cAB"), (tps, "tr"))
            early_i = 0

            def proj_unit(kind, hp, ci, pl, tg):
                l0, ln = L_CHUNKS[ci]
                m = slice(hp * 128, (hp + 1) * 128)
                if kind == "q":
                    qps = pl.tile([128, 512], F32, name="qps", tag=tg)
                    for k in range(KC):
                        nc.tensor.matmul(qps[:, :ln], wq[:, hp, k, :],
                                         xt[:, k, l0:l0 + ln],
                                         start=(k == 0), stop=(k == KC - 1))
                    nc.vector.tensor_scalar_add(qt[:, hp, l0:l0 + ln], qps[:, :ln],
                                                bqc[:, hp:hp + 1])
                else:
                    kps = pl.tile([128, 512], F32, name="kps", tag=tg)
                    for k in range(KC):
                        nc.tensor.matmul(kps[:, :ln], wk[:, hp, k, :],
                                         xt[:, k, l0:l0 + ln],
                                         start=(k == 0), stop=(k == KC - 1))
                    nc.vector.tensor_copy(kt[:, hp, l0:l0 + ln], kps[:, :ln])

            # Startup: the first exp gates on K-c0 + Q-c0 only, so emit
            # those two first (PE is in-order: anything emitted between
            # them and the first score matmul delays the first exp). V0
            # (needed by the first ctx matmul) follows, then K-c1/K-c2
            # whose real deadlines are S-tiles 4 and 8.
            proj_unit("k", 0, 0, *EARLY[0])
            proj_unit("q", 0, 0, *EARLY[1])

            def v_tile(si, pl, tg):
                s0, ss = TILES[si]
                vps = pl.tile([128, QD], F32, name="vps", tag=tg)
                for k in range(KC):
                    nc.tensor.matmul(vps[:ss, :], xt[:, k, s0:s0 + ss], wv[:, k, :],
                                     start=(k == 0), stop=(k == KC - 1))
                nc.vector.tensor_add(
                    vv[:ss, si, :, 0:D],
                    vps[:ss, :].rearrange("p (h d) -> p h d", h=NH),
                    bvb[:ss, :].rearrange("p (h d) -> p h d", h=NH),
                )

            # V tile 0 right behind the startup projections; tiles 1..10 are
            # pipelined one-per-S-tile-slot into head-pair 0's first chunk
            # with one-slot lookahead (tile si+1 is issued in slot si, so its
            # eviction lands well before the ctx matmul that reads it). hp0's
            # Q1/Q2 lead the later chunks the same way. All of these rotate
            # on the proj bank, whose chain pace matches the ACT slot pace.
            v_tile(0, *EARLY[2])
            proj_unit("k", 0, 1, *EARLY[3])
            proj_unit("k", 0, 2, *EARLY[0])

            def pu(kind, h, ci):
                return lambda: proj_unit(kind, h, ci, pqp, "pq")

            # Slot-pipelined deferred work: V tiles 1..10 plus hp0's Q1 fill
            # chunk 0's slots; each later head-pair's six projections spread
            # into the previous head-pair's last chunk. Every unit lands
            # well before its first reader.
            slot_work = {
                (0, 0): [(lambda s: lambda: v_tile(s, pqp, "pq"))(s)
                         for s in range(1, NS)] + [pu("q", 0, 1)],
                (0, 1): [pu("q", 0, 2)],
            }
            for hp in range(1, HP):
                slot_work[(hp - 1, 2)] = [
                    pu("k", hp, 0), pu("k", hp, 1), pu("k", hp, 2),
                    pu("q", hp, 0), pu("q", hp, 1), pu("q", hp, 2)]

            def emit_scores(hp, ci, si):
                l0, ln = L_CHUNKS[ci]
                s0, ss = TILES[si]
                stAB = sps.tile([128, 2, 512], F32, name="stAB", tag="stAB")
                nc.tensor.matmul(stAB[:ss, 0, :ln], kt[0:64, hp, s0:s0 + ss],
                                 qt[0:64, hp, l0:l0 + ln],
                                 start=True, stop=True, tile_position=(0, 0))
                nc.tensor.matmul(stAB[:ss, 1, :ln], kt[64:128, hp, s0:s0 + ss],
                                 qt[64:128, hp, l0:l0 + ln],
                                 start=True, stop=True, tile_position=(64, 0))
                return stAB

            backlog = []
            pre_scored = {}
            CH = [(hp, ci) for hp in range(HP) for ci in range(len(L_CHUNKS))]
            for ch_i, (hp, ci) in enumerate(CH):
                hA, hB = 2 * hp, 2 * hp + 1
                if True:
                    l0, ln = L_CHUNKS[ci]
                    slots = list(slot_work.get((hp, ci), []))
                    cAB = cps.tile([65, 2, 512], F32, name="cAB", tag="cAB")
                    for si, (s0, ss) in enumerate(TILES):
                        # Early output DMAs: L-tiles 0-3 are fully normalized
                        # once (hp3, c1) drained (hp3, c0)'s backlog, and
                        # tiles 4-7 once (hp3, c2)'s slots 0-7 drained
                        # (hp3, c1)'s -- emitting their out-DMAs here (on the
                        # idle sync/gpsimd queues, off the ACT sequencer)
                        # overlaps the DGE configs and transfers with the
                        # final chunk's compute, shrinking the kernel tail to
                        # just tiles 8-10.
                        if hp == HP - 1 and ci == 2 and si in (0, 9):
                            for ti in range(4) if si == 0 else range(4, 8):
                                t0_, tn_ = TILES[ti]
                                q = nc.sync if ti % 2 == 0 else nc.gpsimd
                                q.dma_start(out_d[t0_:t0_ + tn_, :],
                                            ost[:tn_, ti, :])
                        # One pipelined projection/V unit per S-tile slot,
                        # emitted BEFORE the slot's score matmuls (a chunk's
                        # Q projection must precede the first score matmul
                        # that reads it -- reads take no dependency on
                        # later-emitted writes).
                        if slots:
                            slots.pop(0)()
                        if si == 0 and (hp, ci) in pre_scored:
                            stAB = pre_scored.pop((hp, ci))
                        else:
                            stAB = emit_scores(hp, ci, si)
                        # ... and one deferred transpose+normalize unit from
                        # the previous chunk: keeps the hp/chunk boundary
                        # chain off ACT's critical path.
                        if backlog:
                            backlog.pop(0)(tps, "tr")
                        if si == NS - 1 and ch_i + 1 < len(CH):
                            # Software-pipeline the chunk boundary: the next
                            # chunk's first score matmuls go ahead of this
                            # chunk's last exp, so ACT never waits for them.
                            # (Their qt/kt producers were slot-emitted at
                            # least one chunk earlier.)
                            pre_scored[CH[ch_i + 1]] = emit_scores(
                                *CH[ch_i + 1], 0)
                        eAB = wp.tile([128, 2, 512], DT, name="eAB", tag="eAB")
                        nc.scalar.activation(eAB[:ss, :, :ln], stAB[:ss, :, :ln],
                                             AF.Exp, scale=0.125)
                        nc.tensor.matmul(cAB[:, 0, :ln], vv[:ss, si, hA, :],
                                         eAB[:ss, 0, :ln],
                                         start=(si == 0), stop=(si == NS - 1))
                        nc.tensor.matmul(cAB[:, 1, :ln], vv[:ss, si, hB, :],
                                         eAB[:ss, 1, :ln],
                                         start=(si == 0), stop=(si == NS - 1))
                    ctAB = wp.tile([65, 2, 512], DT, name="ctAB", tag="ctAB")
                    nc.vector.tensor_copy(ctAB[:, :, :ln], cAB[:, :, :ln])

                    def norm_unit(ctAB, l0, ln, j, h2, h):
                        def f(pl, tg):
                            lt = (l0 + j) // 128
                            w = min(128, ln - j)
                            tr = pl.tile([128, 65], DT, name="tr", tag=tg)
                            nc.tensor.transpose(tr[:w, :], ctAB[:, h2, j:j + w],
                                                identb[0:65, 0:65])
                            rc = wp.tile([128, 1], F32, name="rc", tag="rc")
                            nc.vector.reciprocal(rc[:w, :], tr[:w, 64:65])
                            nc.vector.tensor_scalar_mul(
                                ost[:w, lt, h * D:(h + 1) * D],
                                tr[:w, 0:D], rc[:w, :])
                        return f

                    for j in range(0, ln, 128):
                        for h2, h in ((0, hA), (1, hB)):
                            backlog.append(norm_unit(ctAB, l0, ln, j, h2, h))

            # Flush the final chunk's normalize units through the now-free
            # attention banks. Tiles 0-7's out-DMAs were emitted inside
            # (hp3, c2)'s slot loop; tiles 8-10 complete pairwise here, so
            # emit each one's DMA right after its second unit to overlap
            # the last DGE configs/transfers with the remaining flush.
            oqs = [nc.sync, nc.gpsimd, nc.sync]
            for i, f in enumerate(backlog):
                pl, tg = ((tps, "tr"), (pqp, "pq"),
                          (cps, "cAB"), (sps, "stAB"))[i % 4]
                f(pl, tg)
                if i % 2 == 1:
                    ti = 8 + i // 2
                    t0, tn = TILES[ti]
                    oqs[i // 2].dma_start(out_d[t0:t0 + tn, :],
                                          ost[:tn, ti, :])
            backlog.clear()


_NC_CACHE = {}


def _build(reps=1):
    key = ("nc", reps)
    if key in _NC_CACHE:
        return _NC_CACHE[key]
    nc = bacc.Bacc("TRN2", target_bir_lowering=False, debug=False)
    xt_d = nc.dram_tensor("xt", [HID, L], DT, kind="ExternalInput")
    wq_d = nc.dram_tensor("wqt", [HP * HID, 128], DT, kind="ExternalInput")
    wk_d = nc.dram_tensor("wkt", [HP * HID, 128], DT, kind="ExternalInput")
    wv_d = nc.dram_tensor("wvt", [HID, QD], DT, kind="ExternalInput")
    bq_d = nc.dram_tensor("bq", [QD, 1], F32, kind="ExternalInput")
    bv_d = nc.dram_tensor("bvb", [128, QD], F32, kind="ExternalInput")
    out_d = nc.dram_tensor("out", [L, QD], F32, kind="ExternalOutput")

    with tile.TileContext(nc) as tc:
        for _ in range(reps):
            _body(nc, tc, xt_d.ap(), wq_d.ap(), wk_d.ap(), wv_d.ap(),
                  bq_d.ap(), bv_d.ap(), out_d.ap())
    nc.compile()
    _NC_CACHE[key] = nc
    return nc


def make_in_maps(hidden_states, Wq, bq, Wk, bk, Wv, bv):
    in_maps = []
    for c in range(8):
        b, g = divmod(c, 2)
        gs = slice(g * QD, (g + 1) * QD)
        in_maps.append({
            "xt": np.ascontiguousarray(hidden_states[b].T).astype(NPDT),
            "wqt": np.ascontiguousarray(
                Wq[gs, :].T.reshape(KC, 128, HP, 128).transpose(2, 1, 0, 3)
                .reshape(HP * HID, 128)).astype(NPDT),
            "wkt": np.ascontiguousarray(
                Wk[gs, :].T.reshape(KC, 128, HP, 128).transpose(2, 1, 0, 3)
                .reshape(HP * HID, 128)).astype(NPDT),
            "wvt": np.ascontiguousarray(Wv[gs, :].T).astype(NPDT),
            "bq": bq[gs].reshape(QD, 1).astype(np.float32),
            "bvb": np.ascontiguousarray(
                np.broadcast_to(bv[gs], (128, QD))).astype(np.float32),
        })
    return in_maps


LAST_RESULTS = None


def kernel(hidden_states, Wq, bq, Wk, bk, Wv, bv):
    global LAST_RESULTS
    nc = _build()
    in_maps = make_in_maps(hidden_states, Wq, bq, Wk, bk, Wv, bv)
    try:
        res = bass_utils.run_bass_kernel_spmd(
            nc, in_maps, core_ids=list(range(8)),
            trace=bool(os.environ.get("KERNEL_TRACE")),
        )
    except (ImportError, ModuleNotFoundError):
        # The axon NTFF profiling hook is absent in some containers; retry
        # with tracing disabled rather than failing the run.
        prev = os.environ.get("BASS_NEVER_TRACE")
        os.environ["BASS_NEVER_TRACE"] = "1"
        try:
            res = bass_utils.run_bass_kernel_spmd(
                nc, in_maps, core_ids=list(range(8)))
        finally:
            if prev is None:
                os.environ.pop("BASS_NEVER_TRACE", None)
            else:
                os.environ["BASS_NEVER_TRACE"] = prev
    LAST_RESULTS = res
    out = np.empty((B, L, HID), np.float32)
    for c, om in enumerate(res.results):
        b, g = divmod(c, 2)
        out[b, :, g * QD:(g + 1) * QD] = om["out"]
    return out



# revision 23
# speedup vs baseline: 1.3318x; 1.1803x over previous
"""Dinov2 SDPA self-attention on one TRN2 chip (8 NeuronCores).

Problem: hidden_states [4, 1370, 1024], 16 heads x 64 dim, fp32.

Sharding (hybrid data/tensor parallel): core c handles batch b = c//2 and
head-group g = c%2 (8 heads = 512 hidden columns). Each core computes its
Q/K/V projections from its batch's activations and runs attention for its
8 heads; the host concatenates the per-core [1370, 512] context outputs.
No on-chip collectives needed.

Per-core algorithm (all layouts transposed so softmax reductions become
matmul contractions):
  Xt = X^T in SBUF  [1024, 1370]
  Qt = Wq_g @ Xt + bq (per-partition bias)   [512, 1370]
  Kt = Wk_g @ Xt  (K bias is softmax-invariant -> dropped exactly)
  V  = X @ Wv_g^T + bv (natural layout, bias via DVE add; softmax weights
       sum to 1 so including bv here is exact)
  per head h: ST = Kt_h^T-tiles @ Qt_h = scores^T  [S, L] (contraction d=64;
       the head pair is emitted as PE row groups 0-63/64-127, though with
       M=128 outputs the PSUM write port serializes them anyway)
  P^T = exp(ST/8) (ACT, fused 1/sqrt(d) scale; no max-subtraction -- scores
       are bounded ~|4|, exp is safe)
  ctxT_ext = [V_h | 1]^T-style stationary @ P^T = [ctx^T; rowsums]  [65, L]
  PE-transpose 128-col slices -> [L_tile, 65], normalize by column 64 via
  DVE reciprocal + per-partition tensor_scalar multiply -> out staging.

Matmul operands are bf16 (fp32 PSUM accumulation); the ctx eviction and
transposes are bf16 too (rounds ctx and rowsums independently: measured
5.7e-3 vs 3.5e-3 with fp32 eviction -- both far under the gate, and bf16
halves the PE transpose cost). fp8/DoubleRow variants measured 2.4e-2..4.7e-2
relative error (quantization noise in a K-dim dot product grows with
sqrt(K) like the signal) -- over the 2e-2 gate, so bf16 throughout.
Validated vs fp32 reference: 5.7e-3 of absmax.

Scheduling (the kernel is exp-bound on hardware, so ACT idle is the enemy):
  - head-pair 0's K projections (all chunks) + Q (chunk 0) are emitted
    first and gate the first exp. wq/wk are hp-major in DRAM so only
    head-pair 0's 256KB slices sit on the startup DMA path (the other
    three head-pairs' slices, first read ~35us in, load later); xt chunks
    lead all three queues; wv trails.
  - all other producer work is slot-pipelined into the attention stream,
    one unit per S-tile slot, emitted at the TOP of the slot (a read takes
    no dependency on a later-emitted write): V tiles 1..10 + Q1 fill
    head-pair 0's first chunk (one-slot lookahead so each V eviction lands
    before the ctx matmul that reads it); each later head-pair's six
    projections spread into the previous head-pair's last chunk.
  - the transpose+normalize units of each chunk are deferred into the NEXT
    chunk's S-tile loop, and each chunk's FIRST score matmuls are emitted
    ahead of the previous chunk's last exp, so the hp/chunk-boundary
    serial chain never idles ACT; the final chunk's normalize units flush
    through the freed attention banks.
  - output DMAs for L-tiles 0-7 are emitted inside head-pair 3's last
    chunk (each right after its final normalize unit has drained), so
    their DGE configs and transfers overlap compute; only tiles 8-10
    remain in the kernel tail. Startup emits K-c0 + Q-c0 (the first
    exp's only gates) before V0/K-c1/K-c2.
"""

import os

import numpy as np
import ml_dtypes

import concourse.mybir as mybir
import concourse.tile as tile
from concourse import bacc
from concourse import bass_utils
from concourse.masks import make_identity

F32 = mybir.dt.float32
DT = mybir.dt.bfloat16
I16 = mybir.dt.int16
NPDT = ml_dtypes.bfloat16
AF = mybir.ActivationFunctionType

# Schraudolph exp on the DVE (i16 = round(s*SFAC+SBIAS) bitcast bf16,
# ~±3.3% sawtooth; softmax cancels the multiplicative part). Used only in
# the ACT-bound second half (head-pairs 2-3 have no projection units left,
# so the PE idles and ACT is the regional bottleneck there).
LOG2E = 1.4426950408889634
SFAC = 0.125 * 128.0 * LOG2E
SBIAS = (127.0 - 0.043) * 128.0

B = 4
L = 1370
HID = 1024
NH = 8            # heads per core
D = 64
QD = NH * D       # 512 projected dims per core
HP = NH // 2      # head pairs (PE row-group packing)
KC = HID // 128   # contraction chunks for projections

L_CHUNKS = [(0, 512), (512, 512), (1024, 346)]                      # moving/free dim
TILES = [(i * 128, min(128, L - i * 128)) for i in range((L + 127) // 128)]
NS = len(TILES)   # 11 (last tile 90)


def _body(nc, tc, xt_d, wq_d, wk_d, wv_d, bq_d, bv_d, out_d):
    with tc.tile_pool(name="persist", bufs=1) as pp:
        xt = pp.tile([128, KC, L], DT)
        wq = pp.tile([128, HP, KC, 128], DT)
        wk = pp.tile([128, HP, KC, 128], DT)
        wv = pp.tile([128, KC, QD], DT)
        qt = pp.tile([128, HP, L], DT)
        kt = pp.tile([128, HP, L], DT)
        vv = pp.tile([128, NS, NH, D + 1], DT)   # V tiles + ones column
        ost = pp.tile([128, NS, QD], F32)        # output staging, natural layout
        bqc = pp.tile([128, HP], F32)
        bvb = pp.tile([128, QD], F32)
        ident = pp.tile([128, 128], F32)
        identb = pp.tile([128, 128], DT)

        make_identity(nc, ident[:, :])
        make_identity(nc, identb[:, :])
        nc.vector.memset(vv[:, :, :, D:D + 1], 1.0)

        # Input DMAs: the startup K/Q projections need every xt chunk plus
        # wk/wq, so those are chunked and spread across the three DGE queues
        # with the gating chunks first; wv follows (first needed by the
        # V-projection phase a few microseconds in).
        xtr = [slice(k * 128, (k + 1) * 128) for k in range(KC)]

        def half(t_d, lo, hi):
            return t_d[lo * 128:hi * 128, :].rearrange("(k p) n -> p k n", p=128)

        # wq/wk are hp-major in DRAM ([HP*HID, 128]): head-pair 0's
        # 256KB slices load first (they gate the first exp); the other three
        # head-pairs' slices (first read ~35us in) are deferred.
        nc.gpsimd.dma_start(wk[:, 0, :, :],
                            wk_d[0:HID, :].rearrange("(p k) n -> p k n", p=128))
        nc.sync.dma_start(xt[:, 0, :], xt_d[xtr[0], :])
        nc.scalar.dma_start(xt[:, 1, :], xt_d[xtr[1], :])
        nc.sync.dma_start(xt[:, 3, :], xt_d[xtr[3], :])
        nc.scalar.dma_start(xt[:, 4, :], xt_d[xtr[4], :])
        nc.sync.dma_start(xt[:, 6, :], xt_d[xtr[6], :])
        nc.scalar.dma_start(xt[:, 7, :], xt_d[xtr[7], :])
        nc.gpsimd.dma_start(xt[:, 2, :], xt_d[xtr[2], :])
        nc.gpsimd.dma_start(xt[:, 5, :], xt_d[xtr[5], :])
        nc.scalar.dma_start(wq[:, 0, :, :],
                            wq_d[0:HID, :].rearrange("(p k) n -> p k n", p=128))
        nc.gpsimd.dma_start(bqc[:, :], bq_d.rearrange("(h p) o -> p (h o)", p=128))
        nc.gpsimd.dma_start(bvb[:, :], bv_d[:, :])
        nc.sync.dma_start(wv[:, 0:4, :], half(wv_d, 0, 4))
        nc.scalar.dma_start(wv[:, 4:KC, :], half(wv_d, 4, KC))
        nc.sync.dma_start(wk[:, 1:HP, :, :], wk_d[HID:HP * HID, :].rearrange(
            "(h p k) n -> p h k n", p=128, k=KC))
        nc.scalar.dma_start(wq[:, 1:HP, :, :], wq_d[HID:HP * HID, :].rearrange(
            "(h p k) n -> p h k n", p=128, k=KC))

        # ---- fused projection + attention ----
        # One concurrent PSUM layout (8 banks: pq 1 + stAB 2x2 + cAB 2 + tr 1)
        # so Q/K projections for later head pairs and the PE-transposes hide
        # inside the exp-bound attention window of earlier head pairs.
        with (
            tc.tile_pool(name="pqp", bufs=1, space="PSUM") as pqp,
            tc.tile_pool(name="sps", bufs=2, space="PSUM") as sps,
            tc.tile_pool(name="cps", bufs=1, space="PSUM") as cps,
            tc.tile_pool(name="tps", bufs=1, space="PSUM") as tps,
            tc.tile_pool(name="wp", bufs=3) as wp,
        ):
            # Head pipelining: the attention banks are idle until hp0's
            # attention starts, so early projection groups rotate through
            # them instead of serializing on the single proj slot.
            EARLY = ((pqp, "pq"), (sps, "stAB"), (cps, "cAB"), (tps, "tr"))
            early_i = 0

            def proj_unit(kind, hp, ci, pl, tg):
                l0, ln = L_CHUNKS[ci]
                m = slice(hp * 128, (hp + 1) * 128)
                if kind == "q":
                    qps = pl.tile([128, 512], F32, name="qps", tag=tg)
                    for k in range(KC):
                        nc.tensor.matmul(qps[:, :ln], wq[:, hp, k, :],
                                         xt[:, k, l0:l0 + ln],
                                         start=(k == 0), stop=(k == KC - 1))
                    nc.vector.tensor_scalar_add(qt[:, hp, l0:l0 + ln], qps[:, :ln],
                                                bqc[:, hp:hp + 1])
                else:
                    kps = pl.tile([128, 512], F32, name="kps", tag=tg)
                    for k in range(KC):
                        nc.tensor.matmul(kps[:, :ln], wk[:, hp, k, :],
                                         xt[:, k, l0:l0 + ln],
                                         start=(k == 0), stop=(k == KC - 1))
                    nc.vector.tensor_copy(kt[:, hp, l0:l0 + ln], kps[:, :ln])

            # Startup: the first exp gates on K-c0 + Q-c0 only, so emit
            # those two first (PE is in-order: anything emitted between
            # them and the first score matmul delays the first exp). V0
            # (needed by the first ctx matmul) follows, then K-c1/K-c2
            # whose real deadlines are S-tiles 4 and 8.
            proj_unit("k", 0, 0, *EARLY[0])
            proj_unit("q", 0, 0, *EARLY[1])

            def v_tile(si, pl, tg):
                s0, ss = TILES[si]
                vps = pl.tile([128, QD], F32, name="vps", tag=tg)
                for k in range(KC):
                    nc.tensor.matmul(vps[:ss, :], xt[:, k, s0:s0 + ss], wv[:, k, :],
                                     start=(k == 0), stop=(k == KC - 1))
                nc.vector.tensor_add(
                    vv[:ss, si, :, 0:D],
                    vps[:ss, :].rearrange("p (h d) -> p h d", h=NH),
                    bvb[:ss, :].rearrange("p (h d) -> p h d", h=NH),
                )

            # V tile 0 right behind the startup projections; tiles 1..10 are
            # pipelined one-per-S-tile-slot into head-pair 0's first chunk
            # with one-slot lookahead (tile si+1 is issued in slot si, so its
            # eviction lands well before the ctx matmul that reads it). hp0's
            # Q1/Q2 lead the later chunks the same way. All of these rotate
            # on the proj bank, whose chain pace matches the ACT slot pace.
            v_tile(0, *EARLY[2])
            proj_unit("k", 0, 1, *EARLY[3])
            proj_unit("k", 0, 2, *EARLY[0])

            def pu(kind, h, ci):
                return lambda: proj_unit(kind, h, ci, pqp, "pq")

            # Slot-pipelined deferred work: V tiles 1..10 plus hp0's Q1 fill
            # chunk 0's slots; each later head-pair's six projections spread
            # into the previous head-pair's last chunk. Every unit lands
            # well before its first reader.
            slot_work = {
                (0, 0): [(lambda s: lambda: v_tile(s, pqp, "pq"))(s)
                         for s in range(1, NS)] + [pu("q", 0, 1)],
                (0, 1): [pu("q", 0, 2)],
            }
            for hp in range(1, HP):
                slot_work[(hp - 1, 2)] = [
                    pu("k", hp, 0), pu("k", hp, 1), pu("k", hp, 2),
                    pu("q", hp, 0), pu("q", hp, 1), pu("q", hp, 2)]

            def emit_scores(hp, ci, si):
                l0, ln = L_CHUNKS[ci]
                s0, ss = TILES[si]
                stAB = sps.tile([128, 2, 512], F32, name="stAB", tag="stAB")
                nc.tensor.matmul(stAB[:ss, 0, :ln], kt[0:64, hp, s0:s0 + ss],
                                 qt[0:64, hp, l0:l0 + ln],
                                 start=True, stop=True, tile_position=(0, 0))
                nc.tensor.matmul(stAB[:ss, 1, :ln], kt[64:128, hp, s0:s0 + ss],
                                 qt[64:128, hp, l0:l0 + ln],
                                 start=True, stop=True, tile_position=(64, 0))
                return stAB

            backlog = []
            pre_scored = {}
            CH = [(hp, ci) for hp in range(HP) for ci in range(len(L_CHUNKS))]
            for ch_i, (hp, ci) in enumerate(CH):
                hA, hB = 2 * hp, 2 * hp + 1
                if True:
                    l0, ln = L_CHUNKS[ci]
                    slots = list(slot_work.get((hp, ci), []))
                    cAB = cps.tile([65, 2, 512], F32, name="cAB", tag="cAB")
                    for si, (s0, ss) in enumerate(TILES):
                        # Early output DMAs: L-tiles 0-3 are fully normalized
                        # once (hp3, c1) drained (hp3, c0)'s backlog, and
                        # tiles 4-7 once (hp3, c2)'s slots 0-7 drained
                        # (hp3, c1)'s -- emitting their out-DMAs here (on the
                        # idle sync/gpsimd queues, off the ACT sequencer)
                        # overlaps the DGE configs and transfers with the
                        # final chunk's compute, shrinking the kernel tail to
                        # just tiles 8-10.
                        if hp == HP - 1 and ci == 2 and si in (0, 9):
                            for ti in range(4) if si == 0 else range(4, 8):
                                t0_, tn_ = TILES[ti]
                                q = nc.sync if ti % 2 == 0 else nc.gpsimd
                                q.dma_start(out_d[t0_:t0_ + tn_, :],
                                            ost[:tn_, ti, :])
                        # One pipelined projection/V unit per S-tile slot,
                        # emitted BEFORE the slot's score matmuls (a chunk's
                        # Q projection must precede the first score matmul
                        # that reads it -- reads take no dependency on
                        # later-emitted writes).
                        if slots:
                            slots.pop(0)()
                        if si == 0 and (hp, ci) in pre_scored:
                            stAB = pre_scored.pop((hp, ci))
                        else:
                            stAB = emit_scores(hp, ci, si)
                        # ... and one deferred transpose+normalize unit from
                        # the previous chunk: keeps the hp/chunk boundary
                        # chain off ACT's critical path.
                        if backlog:
                            backlog.pop(0)(tps, "tr")
                        if si == NS - 1 and ch_i + 1 < len(CH):
                            # Software-pipeline the chunk boundary: the next
                            # chunk's first score matmuls go ahead of this
                            # chunk's last exp, so ACT never waits for them.
                            # (Their qt/kt producers were slot-emitted at
                            # least one chunk earlier.)
                            pre_scored[CH[ch_i + 1]] = emit_scores(
                                *CH[ch_i + 1], 0)
                        eAB = wp.tile([128, 2, 512], DT, name="eAB", tag="eAB")
                        if ch_i >= 6 and si in (3, 7):
                            nc.vector.tensor_scalar(
                                eAB[:ss, :, :ln].bitcast(I16),
                                stAB[:ss, :, :ln], SFAC, SBIAS,
                                mybir.AluOpType.mult, mybir.AluOpType.add)
                        else:
                            nc.scalar.activation(eAB[:ss, :, :ln],
                                                 stAB[:ss, :, :ln],
                                                 AF.Exp, scale=0.125)
                        nc.tensor.matmul(cAB[:, 0, :ln], vv[:ss, si, hA, :],
                                         eAB[:ss, 0, :ln],
                                         start=(si == 0), stop=(si == NS - 1))
                        nc.tensor.matmul(cAB[:, 1, :ln], vv[:ss, si, hB, :],
                                         eAB[:ss, 1, :ln],
                                         start=(si == 0), stop=(si == NS - 1))
                    ctAB = wp.tile([65, 2, 512], DT, name="ctAB", tag="ctAB")
                    nc.vector.tensor_copy(ctAB[:, :, :ln], cAB[:, :, :ln])

                    def norm_unit(ctAB, l0, ln, j, hA, hB):
                        def f(pl, tg):
                            lt = (l0 + j) // 128
                            w = min(128, ln - j)
                            # Both heads per unit: [trA 0:65 | pad | trB
                            # 66:131] (66 so trB's byte offset is 4-aligned)
                            # -> ONE strided 2-element reciprocal replaces
                            # two fixed-cost-dominated [w,1] recips.
                            tr = pl.tile([128, 132], DT, name="tr", tag=tg)
                            nc.tensor.transpose(tr[:w, 0:65],
                                                ctAB[:, 0, j:j + w],
                                                identb[0:65, 0:65])
                            nc.tensor.transpose(tr[:w, 66:131],
                                                ctAB[:, 1, j:j + w],
                                                identb[0:65, 0:65])
                            rc = wp.tile([128, 2], F32, name="rc", tag="rc")
                            nc.vector.reciprocal(rc[:w, :], tr[:w, 64:131:66])
                            nc.vector.tensor_scalar_mul(
                                ost[:w, lt, hA * D:(hA + 1) * D],
                                tr[:w, 0:D], rc[:w, 0:1])
                            nc.vector.tensor_scalar_mul(
                                ost[:w, lt, hB * D:(hB + 1) * D],
                                tr[:w, 66:66 + D], rc[:w, 1:2])
                        return f

                    for j in range(0, ln, 128):
                        backlog.append(norm_unit(ctAB, l0, ln, j, hA, hB))

            # Flush the final chunk's normalize units through the now-free
            # attention banks. Tiles 0-7's out-DMAs were emitted inside
            # (hp3, c2)'s slot loop; tiles 8-10 complete pairwise here, so
            # emit each one's DMA right after its second unit to overlap
            # the last DGE configs/transfers with the remaining flush.
            oqs = [nc.sync, nc.gpsimd, nc.sync]
            for i, f in enumerate(backlog):
                pl, tg = ((tps, "tr"), (pqp, "pq"),
                          (cps, "cAB"), (sps, "stAB"))[i % 4]
                f(pl, tg)
                ti = 8 + i
                if ti < NS:
                    t0, tn = TILES[ti]
                    oqs[i % 3].dma_start(out_d[t0:t0 + tn, :],
                                         ost[:tn, ti, :])
            backlog.clear()


_NC_CACHE = {}


def _build(reps=1):
    key = ("nc", reps)
    if key in _NC_CACHE:
        return _NC_CACHE[key]
    nc = bacc.Bacc("TRN2", target_bir_lowering=False, debug=False)
    xt_d = nc.dram_tensor("xt", [HID, L], DT, kind="ExternalInput")
    wq_d = nc.dram_tensor("wqt", [HP * HID, 128], DT, kind="ExternalInput")
    wk_d = nc.dram_tensor("wkt", [HP * HID, 128], DT, kind="ExternalInput")
    wv_d = nc.dram_tensor("wvt", [HID, QD], DT, kind="ExternalInput")
    bq_d = nc.dram_tensor("bq", [QD, 1], F32, kind="ExternalInput")
    bv_d = nc.dram_tensor("bvb", [128, QD], F32, kind="ExternalInput")
    out_d = nc.dram_tensor("out", [L, QD], F32, kind="ExternalOutput")

    with tile.TileContext(nc) as tc:
        for _ in range(reps):
            _body(nc, tc, xt_d.ap(), wq_d.ap(), wk_d.ap(), wv_d.ap(),
                  bq_d.ap(), bv_d.ap(), out_d.ap())
    nc.compile()
    _NC_CACHE[key] = nc
    return nc


def make_in_maps(hidden_states, Wq, bq, Wk, bk, Wv, bv):
    in_maps = []
    for c in range(8):
        b, g = divmod(c, 2)
        gs = slice(g * QD, (g + 1) * QD)
        in_maps.append({
            "xt": np.ascontiguousarray(hidden_states[b].T).astype(NPDT),
            "wqt": np.ascontiguousarray(
                Wq[gs, :].T.reshape(KC, 128, HP, 128).transpose(2, 1, 0, 3)
                .reshape(HP * HID, 128)).astype(NPDT),
            "wkt": np.ascontiguousarray(
                Wk[gs, :].T.reshape(KC, 128, HP, 128).transpose(2, 1, 0, 3)
                .reshape(HP * HID, 128)).astype(NPDT),
            "wvt": np.ascontiguousarray(Wv[gs, :].T).astype(NPDT),
            "bq": bq[gs].reshape(QD, 1).astype(np.float32),
            "bvb": np.ascontiguousarray(
                np.broadcast_to(bv[gs], (128, QD))).astype(np.float32),
        })
    return in_maps


LAST_RESULTS = None


def kernel(hidden_states, Wq, bq, Wk, bk, Wv, bv):
    global LAST_RESULTS
    nc = _build()
    in_maps = make_in_maps(hidden_states, Wq, bq, Wk, bk, Wv, bv)
    try:
        res = bass_utils.run_bass_kernel_spmd(
            nc, in_maps, core_ids=list(range(8)),
            trace=bool(os.environ.get("KERNEL_TRACE")),
        )
    except (ImportError, ModuleNotFoundError):
        # The axon NTFF profiling hook is absent in some containers; retry
        # with tracing disabled rather than failing the run.
        prev = os.environ.get("BASS_NEVER_TRACE")
        os.environ["BASS_NEVER_TRACE"] = "1"
        try:
            res = bass_utils.run_bass_kernel_spmd(
                nc, in_maps, core_ids=list(range(8)))
        finally:
            if prev is None:
                os.environ.pop("BASS_NEVER_TRACE", None)
            else:
                os.environ["BASS_NEVER_TRACE"] = prev
    LAST_RESULTS = res
    out = np.empty((B, L, HID), np.float32)
    for c, om in enumerate(res.results):
        b, g = divmod(c, 2)
        out[b, :, g * QD:(g + 1) * QD] = om["out"]
    return out

